# revision 4
# baseline (speedup 1.0000x reference)
"""Trainium2 Bass kernel for CLDOdeBlock v2 (fp8 DoubleRow agg + bf16 MLP).

Math (per batch b):
    An = adjacency / max(adjacency.sum(-1, keepdims=True), 1)   (row sums == 1)
    vector_field(t, h) = tanh([h | An@h | te(t)] @ W1 + b1) @ W2 + b2
    RK4 with 8 steps over time_grid; output trajectory [B, T, C, D].

Design (vs the all-f32r baseline):
  - An@h runs in fp8e4 DoubleRow perf mode (0.5 cycles/row, 4x f32r FLOP
    rate): AnT is host-scaled by 512 and stored fp8; h is cast to fp8 on
    device each stage; W1's agg rows are host-scaled by 1/512 to undo. agg
    is a small term (std ~0.04 vs h ~1), so fp8 error on it is negligible.
  - The MLP matmuls (out1, out2) run in bf16 (same 1.0 cycles/row as f32r,
    but evacuations become plain dtype-converting copies that any engine may
    do — no f32r-rounding semantics needed). Only the hT transposes stay
    f32r (their operand is the f32 h state, which IS tf32-rounded because
    the DVE state updates write through f32r-typed APs).
  - b2 is eliminated from the device loop: An is row-stochastic, so constant
    shifts of h pass through aggregation exactly; each stage's b2 shift is
    folded into b1_eff on the host and the stored trajectory is un-biased by
    t_s*b2 in gather().
  - RK4 bookkeeping avoids reading PSUM: with stage states s_g = h + c_g*k_g
    (already computed for the next stage), h_new = u/6 - h/3 + dt/6*k4 where
    u = 2*s1 + 4*s2 + 2*s3. So each p2 PSUM tile has exactly one reader (the
    critical hs update) and its banks recycle immediately; u/z/h' run on DVE
    off the critical path.
  - Phase-major emission (both batches per phase); per-C-quarter hs updates
    so the PE's coalesced wait for the next round's transposes lands just
    after the last quarter.
  - Engine split per eval: PE matmuls; ACT fp8 casts + tanh; Pool hT/agT
    evacuations; DVE state math.
"""

import math
from contextlib import ExitStack, nullcontext

import numpy as np

import concourse.bass as bass
import concourse.tile as tile
from concourse import bacc, mybir
from concourse.bass import ds

B, C, D = 16, 1024, 256
T = 9
NSTEP_FULL = T - 1
NCORES = 8
BPC = B // NCORES  # batches per core
TIME_DIM = 32
HALF = TIME_DIM // 2
F32 = mybir.dt.float32
F32R = mybir.dt.float32r
BF16 = mybir.dt.bfloat16
FP8 = mybir.dt.float8e4

RT = C // 128   # 8 row tiles
DT = D // 128   # 2 feature tiles
NH = C // 512   # 2 free halves for N=512 matmuls
JP = RT // 2    # 4 DoubleRow k-tile pairs
RH = RT // 2    # row tiles per C-half
QD = RT // 4    # row tiles per C-quarter

A_SCALE = 512.0  # host scale on AnT (fp8 range); W1 agg rows divided by it


def build_program(dts, n_steps=NSTEP_FULL, n_iters=1, use_f32r=True,
                  skip_stores=False):
    """Build + compile the per-core Bass program.

    dts: python floats, len n_steps (the RK4 dt per step; baked in).
    n_iters: >1 wraps the whole computation in a For_i loop (for timing).
    """
    nc = bacc.Bacc("TRN2", target_bir_lowering=False, debug=False)

    at_d = nc.dram_tensor("at8", [BPC, RT, 128, C], FP8, kind="ExternalInput").ap()
    h0_d = nc.dram_tensor("h0", [BPC, 128, RT, D], F32, kind="ExternalInput").ap()
    w1_d = nc.dram_tensor("w1", [128, 2, DT, 128], BF16, kind="ExternalInput").ap()
    w1a8_d = nc.dram_tensor("w1a8", [128, 2, DT, 128], FP8, kind="ExternalInput").ap()
    w2_d = nc.dram_tensor("w2", [128, DT, D], BF16, kind="ExternalInput").ap()
    b1_d = nc.dram_tensor("b1t", [128, DT, 4 * NSTEP_FULL], F32, kind="ExternalInput").ap()
    id_d = nc.dram_tensor("ident", [128, 128], F32, kind="ExternalInput").ap()
    tr_d = nc.dram_tensor("traj", [BPC, n_steps, 128, RT, D], F32, kind="ExternalOutput").ap()

    def mm(ap):
        return ap.bitcast(F32R) if use_f32r else ap

    DR = mybir.MatmulPerfMode.DoubleRow

    with ExitStack() as ctx:
        tc = ctx.enter_context(tile.TileContext(nc))
        const = ctx.enter_context(tc.tile_pool(name="const", bufs=1))
        at_p = ctx.enter_context(tc.tile_pool(name="atp", bufs=1))

        # ---- constants / weights ----
        at_sb = at_p.tile([128, BPC, RT, C], FP8)
        for b in range(BPC):
            for jc in range(RT):
                nc.sync.dma_start(at_sb[:, b, jc, :], at_d[b, jc])
        w1_sb = const.tile([128, 2, DT, 128], BF16)
        nc.sync.dma_start(w1_sb[:], w1_d)
        w1a8_sb = const.tile([128, 2, DT, 128], FP8)
        nc.sync.dma_start(w1a8_sb[:], w1a8_d)
        w2_sb = const.tile([128, DT, D], BF16)
        nc.sync.dma_start(w2_sb[:], w2_d)
        b1_sb = const.tile([128, DT, 4 * NSTEP_FULL], F32)
        nc.sync.dma_start(b1_sb[:], b1_d)
        id_sb = const.tile([128, 128], F32)
        nc.sync.dma_start(mm(id_sb[:]), mm(id_d))

        # ---- main pools ----
        state_p = ctx.enter_context(tc.tile_pool(name="state", bufs=4))
        hs_p = ctx.enter_context(tc.tile_pool(name="hs", bufs=2))
        bk_p = ctx.enter_context(tc.tile_pool(name="bk", bufs=2))
        tp_p = ctx.enter_context(tc.tile_pool(name="tp", bufs=4))
        h8_p = ctx.enter_context(tc.tile_pool(name="h8", bufs=3))
        ps_p = ctx.enter_context(tc.tile_pool(name="ps", bufs=4, space="PSUM"))

        loop_cm = tc.For_i(0, n_iters) if n_iters > 1 else nullcontext()
        with loop_cm:
            hstates = []
            for b in range(BPC):
                hst = state_p.tile([128, RT, D], F32, tag="hst")
                nc.sync.dma_start(mm(hst[:]), mm(h0_d[b]))
                hstates.append(hst)
            hstage = [None] * BPC
            pending_bk = []
            zneg = [None] * BPC
            hprime = [None] * BPC

            for s in range(n_steps):
                dt = float(dts[s])
                for g in range(4):
                    ev = s * 4 + g
                    h_in = [hstates[b] if g == 0 else hstage[b] for b in range(BPC)]

                    # --- phase 0: fp8 cast of h, per C-half on Pool
                    # (SBUF->SBUF; Pool cannot access PSUM). The previous
                    # eval's bookkeeping is emitted AFTER these casts so the
                    # Pool queue never head-of-line blocks the agg matmuls. ---
                    h8s = []
                    for b in range(BPC):
                        h8 = h8_p.tile([128, RT, D], FP8, tag="h8")
                        nc.gpsimd.tensor_copy(h8[:, :RH], h_in[b][:, :RH])
                        nc.gpsimd.tensor_copy(h8[:, RH:], h_in[b][:, RH:])
                        h8s.append(h8)
                    for fn in pending_bk:
                        fn()
                    pending_bk = []

                    # --- phase 1: hT transposes (PE, f32r, jc-major), evac
                    # to bf16 on Pool ---
                    hTs = []
                    for b in range(BPC):
                        hT = tp_p.tile([128, DT, C], BF16, tag="tp")
                        pts = [
                            ps_p.tile([128, C], F32, tag="ps", name=f"pt{d_}")
                            for d_ in range(DT)
                        ]
                        for jc in range(RT):
                            for d_ in range(DT):
                                nc.tensor.transpose(
                                    mm(pts[d_][:, ds(jc * 128, 128)]),
                                    mm(h_in[b][:, jc, ds(d_ * 128, 128)]),
                                    mm(id_sb[:]),
                                )
                        for d_ in range(DT):
                            nc.scalar.copy(hT[:, d_, :], pts[d_][:])
                        hTs.append(hT)

                    # --- phase 2: aggT via fp8 DoubleRow (PE), evac to fp8
                    # (x1/256 => agg*2) on DVE ---
                    agTs = []
                    for b in range(BPC):
                        agT = tp_p.tile([128, DT, C], FP8, tag="tp8")
                        for d_ in range(DT):
                            pa = ps_p.tile([128, C], F32, tag="ps")
                            for jp in range(JP):
                                for nh in range(NH):
                                    nc.tensor.matmul(
                                        pa[:, ds(nh * 512, 512)],
                                        h8s[b][:, ds(2 * jp, 2), ds(d_ * 128, 128)],
                                        at_sb[:, b, ds(2 * jp, 2), ds(nh * 512, 512)],
                                        start=(jp == 0),
                                        stop=(jp == JP - 1),
                                        perf_mode=DR,
                                    )
                            nc.vector.tensor_scalar_mul(
                                agT[:, d_, :], pa[:], 1.0 / 256.0
                            )
                        agTs.append(agT)

                    # --- phase 3: out1 (PE, bf16) + tanh w/ folded bias (ACT,
                    # bf16 out) ---
                    a1s = []
                    for b in range(BPC):
                        a1 = tp_p.tile([128, DT, C], BF16, tag="tp")
                        for ht in range(DT):
                            p1 = ps_p.tile([128, C], F32, tag="ps")
                            for kc in range(2):
                                for nh in range(NH):
                                    nc.tensor.matmul(
                                        p1[:, ds(nh * 512, 512)],
                                        w1_sb[:, kc, ht, :],
                                        hTs[b][:, kc, ds(nh * 512, 512)],
                                        start=(kc == 0),
                                        stop=False,
                                        skip_group_check=True,
                                    )
                            for nh in range(NH):
                                nc.tensor.matmul(
                                    p1[:, ds(nh * 512, 512)],
                                    w1a8_sb[:, :, ht, :],
                                    agTs[b][:, :, ds(nh * 512, 512)],
                                    start=False,
                                    stop=True,
                                    perf_mode=DR,
                                    skip_group_check=True,
                                )
                            nc.scalar.activation(
                                a1[:, ht, :],
                                p1[:],
                                mybir.ActivationFunctionType.Tanh,
                                bias=b1_sb[:, ht, ev : ev + 1],
                                scale=1.0 / 16.0,
                            )
                        a1s.append(a1)

                    # --- phase 4: out2 (PE, bf16) + critical hs updates (DVE,
                    # per C-quarter, sole readers of p2) ---
                    hss = []
                    for b in range(BPC):
                        if g < 3:
                            hs = hs_p.tile([128, RT, D], F32, tag="hs")
                        else:
                            hs = state_p.tile([128, RT, D], F32, tag="hst")
                        hss.append(hs)
                        p2s = []
                        for rh in range(2):
                            p2 = ps_p.tile([128, RH, D], F32, tag="ps")
                            p2s.append(p2)
                            for r4 in range(RH):
                                rt = rh * RH + r4
                                for kc in range(DT):
                                    nc.tensor.matmul(
                                        p2[:, r4, :],
                                        a1s[b][:, kc, ds(rt * 128, 128)],
                                        w2_sb[:, kc, :],
                                        start=(kc == 0),
                                        stop=(kc == DT - 1),
                                    )
                        cs = (dt / 2 if g < 2 else dt) if g < 3 else dt / 6.0
                        base = hstates[b] if g < 3 else hprime[b]
                        for q in range(4):
                            hsl = ds(q * QD, QD)
                            p2q = p2s[q // 2][:, ds((q % 2) * QD, QD)]
                            nc.vector.scalar_tensor_tensor(
                                mm(hs[:, hsl]), p2q, cs, base[:, hsl],
                                mybir.AluOpType.mult, mybir.AluOpType.add,
                            )

                    # --- phase 5: bookkeeping on Pool (SBUF only; Pool
                    # supports only TensorTensor/TensorScalar/TensorCopy).
                    # h' = (s1 + 2*s2 + s3 - h)/3, built incrementally in z
                    # with emission deferred to the next eval's phase 0:
                    #   after g0: z = s1 - h
                    #   after g1: z += s2; z += s2
                    #   after g2: z += s3; h' = z/3
                    for b in range(BPC):
                        hs = hss[b]
                        if g == 0:
                            z = bk_p.tile([128, RT, D], F32, tag="z")
                            zneg[b] = z

                            def bk0(z=z, hs=hs, hst=hstates[b]):
                                nc.gpsimd.tensor_tensor(
                                    z[:], hs[:], hst[:],
                                    mybir.AluOpType.subtract,
                                )

                            pending_bk.append(bk0)
                        elif g == 1:

                            def bk1(z=zneg[b], hs=hs):
                                nc.gpsimd.tensor_tensor(
                                    z[:], z[:], hs[:], mybir.AluOpType.add
                                )
                                nc.gpsimd.tensor_tensor(
                                    z[:], z[:], hs[:], mybir.AluOpType.add
                                )

                            pending_bk.append(bk1)
                        elif g == 2:
                            hp = bk_p.tile([128, RT, D], F32, tag="hp")
                            hprime[b] = hp

                            def bk2(z=zneg[b], hs=hs, hp=hp):
                                nc.gpsimd.tensor_tensor(
                                    z[:], z[:], hs[:], mybir.AluOpType.add
                                )
                                nc.gpsimd.tensor_scalar_mul(
                                    hp[:], z[:], 1.0 / 3.0
                                )

                            pending_bk.append(bk2)
                        else:
                            hstates[b] = hss[b]
                            if not skip_stores:
                                nc.sync.dma_start(tr_d[b, s], hss[b][:])
                        if g < 3:
                            hstage[b] = hss[b]

    nc.compile()
    return nc


def tf32_round(x):
    """Round-to-nearest-even to TF32 (10 mantissa bits)."""
    u = np.ascontiguousarray(x, np.float32).view(np.uint32)
    lsb = (u >> np.uint32(13)) & np.uint32(1)
    u = u + np.uint32(0x0FFF) + lsb
    u &= np.uint32(0xFFFFE000)
    return u.view(np.float32)


def host_prep(h0, time_grid, adjacency, W1, b1, W2, b2, n_steps=NSTEP_FULL,
              use_f32r=True):
    """Returns (in_maps list per core, dts list)."""
    import ml_dtypes

    h0 = np.asarray(h0, np.float32)
    time_grid = np.asarray(time_grid, np.float32)
    adjacency = np.asarray(adjacency, np.float32)
    W1 = np.asarray(W1, np.float32)
    b1 = np.asarray(b1, np.float32)
    W2 = np.asarray(W2, np.float32)
    b2 = np.asarray(b2, np.float32)

    # degree normalization (time-constant input prep)
    deg = np.maximum(adjacency.sum(-1, keepdims=True), np.float32(1.0))
    # the b2 fold below needs exact row sums of 1 (degree > 1 everywhere)
    assert adjacency.sum(-1).min() > 1.0
    adjacency = (adjacency / deg).astype(np.float32)

    rnd = tf32_round if use_f32r else (lambda x: x)

    # Fourier features folded into per-eval bias
    freqs = np.exp(
        -math.log(10000.0) * np.arange(HALF, dtype=np.float32) / np.float32(HALF)
    ).astype(np.float32)

    def te(t):
        a = (np.float32(t) * freqs).astype(np.float32)
        return np.concatenate([np.sin(a), np.cos(a)]).astype(np.float32)

    # b2 fold: stored state h~ = h - o(s), o(s) = (t_s - t_0) * b2. Constant
    # shifts pass through the row-stochastic aggregation exactly, so each
    # stage's pre-tanh shift is o_g @ (W1h + W1a), folded into b1_eff.
    W1ha = (W1[:D] + W1[D : 2 * D]).astype(np.float32)  # [D, D]
    t0_grid = np.float32(time_grid[0])

    dts = []
    b1_eff = np.zeros((4 * NSTEP_FULL, D), np.float32)
    for s in range(NSTEP_FULL):
        t0 = np.float32(time_grid[s])
        t1 = np.float32(time_grid[s + 1])
        dt = np.float32(t1 - t0)
        dts.append(float(dt))
        stage_ts = [t0, np.float32(t0 + dt / 2), np.float32(t0 + dt / 2),
                    np.float32(t0 + dt)]
        stage_cs = [np.float32(0.0), np.float32(dt / 2), np.float32(dt / 2), dt]
        for g, (tg_, cg_) in enumerate(zip(stage_ts, stage_cs)):
            o_g = (t0 - t0_grid + cg_) * b2  # offset of stage-g input state
            b1_eff[s * 4 + g] = b1 + te(tg_) @ W1[2 * D:] + o_g @ W1ha

    # Global pre-tanh scale LAMBDA=16, undone by the tanh's scale=1/16:
    # W1h*16 (bf16); agg path: (W1a*8) fp8 x (agg*2) fp8 -> W1a@agg * 16.
    w1_in = np.ascontiguousarray(
        (W1[:D] * np.float32(16.0)).reshape(2, 128, DT, 128).transpose(1, 0, 2, 3)
    ).astype(ml_dtypes.bfloat16)
    w1a8_in = np.ascontiguousarray(
        (W1[D : 2 * D] * np.float32(8.0))
        .reshape(2, 128, DT, 128).transpose(1, 0, 2, 3)
    ).astype(ml_dtypes.float8_e4m3)
    w2_in = np.ascontiguousarray(
        W2.reshape(DT, 128, D).transpose(1, 0, 2)
    ).astype(ml_dtypes.bfloat16)
    b1_in = np.ascontiguousarray(
        b1_eff.reshape(4 * NSTEP_FULL, DT, 128).transpose(2, 1, 0)
    )
    ident = np.eye(128, dtype=np.float32)

    in_maps = []
    for ci in range(NCORES):
        sl = slice(ci * BPC, (ci + 1) * BPC)
        at_in = (
            adjacency[sl].transpose(0, 2, 1).reshape(BPC, RT, 128, C)
            * np.float32(A_SCALE)
        ).astype(ml_dtypes.float8_e4m3)
        h0_in = rnd(np.ascontiguousarray(
            h0[sl].reshape(BPC, RT, 128, D).transpose(0, 2, 1, 3)))
        in_maps.append(
            {
                "at8": at_in,
                "h0": h0_in,
                "w1": w1_in,
                "w1a8": w1a8_in,
                "w2": w2_in,
                "b1t": b1_in,
                "ident": ident,
            }
        )
    return in_maps, dts[:n_steps]


def _bias_table(time_grid, b2):
    # un-bias: true h = stored h~ + (t_s - t_0) * b2
    tg = np.asarray(time_grid, np.float32)
    return (tg[1:, None] - tg[0]) * np.asarray(b2, np.float32)[None, :]  # [T-1, D]


def gather(results, h0, time_grid, b2, n_steps=NSTEP_FULL):
    h0 = np.asarray(h0, np.float32)
    out = np.empty((B, n_steps + 1, C, D), np.float32)
    out[:, 0] = h0
    bias = _bias_table(time_grid, b2)[:n_steps]  # [n_steps, D]
    for ci in range(NCORES):
        t = results[ci]["traj"]  # [BPC, n_steps, 128, RT, D]
        out[ci * BPC : (ci + 1) * BPC, 1:] = (
            t.transpose(0, 1, 3, 2, 4).reshape(BPC, n_steps, C, D)
            + bias[None, :, None, :]
        )
    return out


def postprocess_core0(traj_core0, ins):
    """sim_run helper: apply the gather-side bias to core-0 output."""
    bias = _bias_table(ins["time_grid"], ins["b2"])  # [T-1, D]
    return traj_core0 + bias[None, :, None, :]


_CACHE = {}


def kernel(h0, time_grid, adjacency, W1, b1, W2, b2):
    from concourse.bass_utils import run_bass_kernel_spmd

    in_maps, dts = host_prep(h0, time_grid, adjacency, W1, b1, W2, b2)
    key = tuple(dts)
    if key not in _CACHE:
        _CACHE[key] = build_program(dts)
    nc = _CACHE[key]
    res = run_bass_kernel_spmd(nc, in_maps, list(range(NCORES)), trace=False)
    return gather(res.results, h0, time_grid, b2)


# revision 5
# speedup vs baseline: 7.2509x; 7.2509x over previous
"""Trainium2 Bass kernel for CLDOdeBlock v2 (fp8 DoubleRow agg + bf16 MLP).

Math (per batch b):
    An = adjacency / max(adjacency.sum(-1, keepdims=True), 1)   (row sums == 1)
    vector_field(t, h) = tanh([h | An@h | te(t)] @ W1 + b1) @ W2 + b2
    RK4 with 8 steps over time_grid; output trajectory [B, T, C, D].

Design (vs the all-f32r baseline):
  - An@h runs in fp8e4 DoubleRow perf mode (0.5 cycles/row, 4x f32r FLOP
    rate): AnT is host-scaled by 512 and stored fp8; h is cast to fp8 on
    device each stage; W1's agg rows are host-scaled by 1/512 to undo. agg
    is a small term (std ~0.04 vs h ~1), so fp8 error on it is negligible.
  - The MLP matmuls (out1, out2) run in bf16 (same 1.0 cycles/row as f32r,
    but evacuations become plain dtype-converting copies that any engine may
    do — no f32r-rounding semantics needed). Only the hT transposes stay
    f32r (their operand is the f32 h state, which IS tf32-rounded because
    the DVE state updates write through f32r-typed APs).
  - b2 is eliminated from the device loop: An is row-stochastic, so constant
    shifts of h pass through aggregation exactly; each stage's b2 shift is
    folded into b1_eff on the host and the stored trajectory is un-biased by
    t_s*b2 in gather().
  - RK4 bookkeeping avoids reading PSUM: with stage states s_g = h + c_g*k_g
    (already computed for the next stage), h_new = u/6 - h/3 + dt/6*k4 where
    u = 2*s1 + 4*s2 + 2*s3. So each p2 PSUM tile has exactly one reader (the
    critical hs update) and its banks recycle immediately; u/z/h' run on DVE
    off the critical path.
  - Phase-major emission (both batches per phase); per-C-quarter hs updates
    so the PE's coalesced wait for the next round's transposes lands just
    after the last quarter.
  - Engine split per eval: PE matmuls; ACT fp8 casts + tanh; Pool hT/agT
    evacuations; DVE state math.
"""

import math
from contextlib import ExitStack, nullcontext

import numpy as np

import concourse.bass as bass
import concourse.tile as tile
from concourse import bacc, mybir
from concourse.bass import ds

B, C, D = 16, 1024, 256
T = 9
NSTEP_FULL = T - 1
NCORES = 8
BPC = B // NCORES  # batches per core
TIME_DIM = 32
HALF = TIME_DIM // 2
F32 = mybir.dt.float32
F32R = mybir.dt.float32r
BF16 = mybir.dt.bfloat16
FP8 = mybir.dt.float8e4

RT = C // 128   # 8 row tiles
DT = D // 128   # 2 feature tiles
NH = C // 512   # 2 free halves for N=512 matmuls
JP = RT // 2    # 4 DoubleRow k-tile pairs
RH = RT // 2    # row tiles per C-half
QD = RT // 4    # row tiles per C-quarter

A_SCALE = 512.0  # host scale on AnT (fp8 range); W1 agg rows divided by it


def build_program(dts, n_steps=NSTEP_FULL, n_iters=1, use_f32r=True,
                  skip_stores=False):
    """Build + compile the per-core Bass program.

    dts: python floats, len n_steps (the RK4 dt per step; baked in).
    n_iters: >1 wraps the whole computation in a For_i loop (for timing).
    """
    nc = bacc.Bacc("TRN2", target_bir_lowering=False, debug=False)

    at_d = nc.dram_tensor("at8", [BPC, RT, 128, C], FP8, kind="ExternalInput").ap()
    h0_d = nc.dram_tensor("h0", [BPC, 128, RT, D], F32, kind="ExternalInput").ap()
    w1_d = nc.dram_tensor("w1", [128, 2, DT, 128], BF16, kind="ExternalInput").ap()
    w1a8_d = nc.dram_tensor("w1a8", [128, 2, DT, 128], FP8, kind="ExternalInput").ap()
    w2_d = nc.dram_tensor("w2", [128, DT, D], BF16, kind="ExternalInput").ap()
    b1_d = nc.dram_tensor("b1t", [128, DT, 4 * NSTEP_FULL], F32, kind="ExternalInput").ap()
    id_d = nc.dram_tensor("ident", [128, 128], F32, kind="ExternalInput").ap()
    tr_d = nc.dram_tensor("traj", [BPC, n_steps, 128, RT, D], F32, kind="ExternalOutput").ap()

    def mm(ap):
        return ap.bitcast(F32R) if use_f32r else ap

    DR = mybir.MatmulPerfMode.DoubleRow

    with ExitStack() as ctx:
        tc = ctx.enter_context(tile.TileContext(nc))
        const = ctx.enter_context(tc.tile_pool(name="const", bufs=1))
        at_p = ctx.enter_context(tc.tile_pool(name="atp", bufs=1))

        # ---- constants / weights ----
        at_sb = at_p.tile([128, BPC, RT, C], FP8)
        for b in range(BPC):
            for jc in range(RT):
                nc.sync.dma_start(at_sb[:, b, jc, :], at_d[b, jc])
        w1_sb = const.tile([128, 2, DT, 128], BF16)
        nc.sync.dma_start(w1_sb[:], w1_d)
        w1a8_sb = const.tile([128, 2, DT, 128], FP8)
        nc.sync.dma_start(w1a8_sb[:], w1a8_d)
        w2_sb = const.tile([128, DT, D], BF16)
        nc.sync.dma_start(w2_sb[:], w2_d)
        b1_sb = const.tile([128, DT, 4 * NSTEP_FULL], F32)
        nc.sync.dma_start(b1_sb[:], b1_d)
        id_sb = const.tile([128, 128], F32)
        nc.sync.dma_start(mm(id_sb[:]), mm(id_d))

        # ---- main pools ----
        state_p = ctx.enter_context(tc.tile_pool(name="state", bufs=4))
        hs_p = ctx.enter_context(tc.tile_pool(name="hs", bufs=2))
        bk_p = ctx.enter_context(tc.tile_pool(name="bk", bufs=2))
        tp_p = ctx.enter_context(tc.tile_pool(name="tp", bufs=4))
        h8_p = ctx.enter_context(tc.tile_pool(name="h8", bufs=3))
        ps_p = ctx.enter_context(tc.tile_pool(name="ps", bufs=4, space="PSUM"))

        loop_cm = tc.For_i(0, n_iters) if n_iters > 1 else nullcontext()
        with loop_cm:
            hstates = []
            for b in range(BPC):
                hst = state_p.tile([128, RT, D], F32, tag="hst")
                nc.sync.dma_start(mm(hst[:]), mm(h0_d[b]))
                hstates.append(hst)
            hstage = [None] * BPC
            pending_bk = []
            zneg = [None] * BPC
            hprime = [None] * BPC

            for s in range(n_steps):
                dt = float(dts[s])
                for g in range(4):
                    ev = s * 4 + g
                    h_in = [hstates[b] if g == 0 else hstage[b] for b in range(BPC)]

                    # --- phase 0: fp8 cast of h, per C-half on Pool
                    # (SBUF->SBUF; Pool cannot access PSUM). The previous
                    # eval's bookkeeping is emitted AFTER these casts so the
                    # Pool queue never head-of-line blocks the agg matmuls. ---
                    h8s = []
                    for b in range(BPC):
                        h8 = h8_p.tile([128, RT, D], FP8, tag="h8")
                        nc.scalar.copy(h8[:, :RH], h_in[b][:, :RH])
                        nc.scalar.copy(h8[:, RH:], h_in[b][:, RH:])
                        h8s.append(h8)
                    for fn in pending_bk:
                        fn()
                    pending_bk = []

                    # --- phase 1: hT transposes (PE, f32r, jc-major), evac
                    # to bf16 on Pool ---
                    hTs = []
                    for b in range(BPC):
                        hT = tp_p.tile([128, DT, C], BF16, tag="tp")
                        pts = [
                            ps_p.tile([128, C], F32, tag="ps", name=f"pt{d_}")
                            for d_ in range(DT)
                        ]
                        for jc in range(RT):
                            for d_ in range(DT):
                                nc.tensor.transpose(
                                    mm(pts[d_][:, ds(jc * 128, 128)]),
                                    mm(h_in[b][:, jc, ds(d_ * 128, 128)]),
                                    mm(id_sb[:]),
                                )
                        for d_ in range(DT):
                            nc.scalar.copy(hT[:, d_, :], pts[d_][:])
                        hTs.append(hT)

                    # --- phase 2: aggT via fp8 DoubleRow (PE), evac to fp8
                    # (x1/256 => agg*2) on DVE ---
                    agTs = []
                    for b in range(BPC):
                        agT = tp_p.tile([128, DT, C], FP8, tag="tp8")
                        for d_ in range(DT):
                            pa = ps_p.tile([128, C], F32, tag="ps")
                            for jp in range(JP):
                                for nh in range(NH):
                                    nc.tensor.matmul(
                                        pa[:, ds(nh * 512, 512)],
                                        h8s[b][:, ds(2 * jp, 2), ds(d_ * 128, 128)],
                                        at_sb[:, b, ds(2 * jp, 2), ds(nh * 512, 512)],
                                        start=(jp == 0),
                                        stop=(jp == JP - 1),
                                        perf_mode=DR,
                                    )
                            nc.vector.tensor_scalar_mul(
                                agT[:, d_, :], pa[:], 1.0 / 256.0
                            )
                        agTs.append(agT)

                    # --- phase 3: out1 (PE, bf16) + tanh w/ folded bias (ACT,
                    # bf16 out) ---
                    a1s = []
                    for b in range(BPC):
                        a1 = tp_p.tile([128, DT, C], BF16, tag="tp")
                        for ht in range(DT):
                            p1 = ps_p.tile([128, C], F32, tag="ps")
                            for kc in range(2):
                                for nh in range(NH):
                                    nc.tensor.matmul(
                                        p1[:, ds(nh * 512, 512)],
                                        w1_sb[:, kc, ht, :],
                                        hTs[b][:, kc, ds(nh * 512, 512)],
                                        start=(kc == 0),
                                        stop=False,
                                        skip_group_check=True,
                                    )
                            for nh in range(NH):
                                nc.tensor.matmul(
                                    p1[:, ds(nh * 512, 512)],
                                    w1a8_sb[:, :, ht, :],
                                    agTs[b][:, :, ds(nh * 512, 512)],
                                    start=False,
                                    stop=True,
                                    perf_mode=DR,
                                    skip_group_check=True,
                                )
                            nc.scalar.activation(
                                a1[:, ht, :],
                                p1[:],
                                mybir.ActivationFunctionType.Tanh,
                                bias=b1_sb[:, ht, ev : ev + 1],
                                scale=1.0 / 16.0,
                            )
                        a1s.append(a1)

                    # --- phase 4: out2 (PE, bf16) + critical hs updates (DVE,
                    # per C-quarter, sole readers of p2) ---
                    hss = []
                    for b in range(BPC):
                        if g < 3:
                            hs = hs_p.tile([128, RT, D], F32, tag="hs")
                        else:
                            hs = state_p.tile([128, RT, D], F32, tag="hst")
                        hss.append(hs)
                        p2s = []
                        for rh in range(2):
                            p2 = ps_p.tile([128, RH, D], F32, tag="ps")
                            p2s.append(p2)
                            for r4 in range(RH):
                                rt = rh * RH + r4
                                for kc in range(DT):
                                    nc.tensor.matmul(
                                        p2[:, r4, :],
                                        a1s[b][:, kc, ds(rt * 128, 128)],
                                        w2_sb[:, kc, :],
                                        start=(kc == 0),
                                        stop=(kc == DT - 1),
                                    )
                        cs = (dt / 2 if g < 2 else dt) if g < 3 else dt / 6.0
                        base = hstates[b] if g < 3 else hprime[b]
                        for q in range(4):
                            hsl = ds(q * QD, QD)
                            p2q = p2s[q // 2][:, ds((q % 2) * QD, QD)]
                            nc.vector.scalar_tensor_tensor(
                                mm(hs[:, hsl]), p2q, cs, base[:, hsl],
                                mybir.AluOpType.mult, mybir.AluOpType.add,
                            )

                    # --- phase 5: bookkeeping on Pool (SBUF only; Pool
                    # supports only TensorTensor/TensorScalar/TensorCopy).
                    # h' = (s1 + 2*s2 + s3 - h)/3, built incrementally in z
                    # with emission deferred to the next eval's phase 0:
                    #   after g0: z = s1 - h
                    #   after g1: z += s2; z += s2
                    #   after g2: z += s3; h' = z/3
                    for b in range(BPC):
                        hs = hss[b]
                        if g == 0:
                            z = bk_p.tile([128, RT, D], F32, tag="z")
                            zneg[b] = z

                            def bk0(z=z, hs=hs, hst=hstates[b]):
                                nc.vector.tensor_tensor(
                                    z[:], hs[:], hst[:],
                                    mybir.AluOpType.subtract,
                                )

                            pending_bk.append(bk0)
                        elif g == 1:

                            def bk1(z=zneg[b], hs=hs):
                                nc.vector.tensor_tensor(
                                    z[:], z[:], hs[:], mybir.AluOpType.add
                                )
                                nc.vector.tensor_tensor(
                                    z[:], z[:], hs[:], mybir.AluOpType.add
                                )

                            pending_bk.append(bk1)
                        elif g == 2:
                            hp = bk_p.tile([128, RT, D], F32, tag="hp")
                            hprime[b] = hp

                            def bk2(z=zneg[b], hs=hs, hp=hp):
                                nc.vector.tensor_tensor(
                                    z[:], z[:], hs[:], mybir.AluOpType.add
                                )
                                nc.vector.tensor_scalar_mul(
                                    hp[:], z[:], 1.0 / 3.0
                                )

                            pending_bk.append(bk2)
                        else:
                            hstates[b] = hss[b]
                            if not skip_stores:
                                nc.sync.dma_start(tr_d[b, s], hss[b][:])
                        if g < 3:
                            hstage[b] = hss[b]

    nc.compile()
    return nc


def tf32_round(x):
    """Round-to-nearest-even to TF32 (10 mantissa bits)."""
    u = np.ascontiguousarray(x, np.float32).view(np.uint32)
    lsb = (u >> np.uint32(13)) & np.uint32(1)
    u = u + np.uint32(0x0FFF) + lsb
    u &= np.uint32(0xFFFFE000)
    return u.view(np.float32)


def host_prep(h0, time_grid, adjacency, W1, b1, W2, b2, n_steps=NSTEP_FULL,
              use_f32r=True):
    """Returns (in_maps list per core, dts list)."""
    import ml_dtypes

    h0 = np.asarray(h0, np.float32)
    time_grid = np.asarray(time_grid, np.float32)
    adjacency = np.asarray(adjacency, np.float32)
    W1 = np.asarray(W1, np.float32)
    b1 = np.asarray(b1, np.float32)
    W2 = np.asarray(W2, np.float32)
    b2 = np.asarray(b2, np.float32)

    # degree normalization (time-constant input prep)
    deg = np.maximum(adjacency.sum(-1, keepdims=True), np.float32(1.0))
    # the b2 fold below needs exact row sums of 1 (degree > 1 everywhere)
    assert adjacency.sum(-1).min() > 1.0
    adjacency = (adjacency / deg).astype(np.float32)

    rnd = tf32_round if use_f32r else (lambda x: x)

    # Fourier features folded into per-eval bias
    freqs = np.exp(
        -math.log(10000.0) * np.arange(HALF, dtype=np.float32) / np.float32(HALF)
    ).astype(np.float32)

    def te(t):
        a = (np.float32(t) * freqs).astype(np.float32)
        return np.concatenate([np.sin(a), np.cos(a)]).astype(np.float32)

    # b2 fold: stored state h~ = h - o(s), o(s) = (t_s - t_0) * b2. Constant
    # shifts pass through the row-stochastic aggregation exactly, so each
    # stage's pre-tanh shift is o_g @ (W1h + W1a), folded into b1_eff.
    W1ha = (W1[:D] + W1[D : 2 * D]).astype(np.float32)  # [D, D]
    t0_grid = np.float32(time_grid[0])

    dts = []
    b1_eff = np.zeros((4 * NSTEP_FULL, D), np.float32)
    for s in range(NSTEP_FULL):
        t0 = np.float32(time_grid[s])
        t1 = np.float32(time_grid[s + 1])
        dt = np.float32(t1 - t0)
        dts.append(float(dt))
        stage_ts = [t0, np.float32(t0 + dt / 2), np.float32(t0 + dt / 2),
                    np.float32(t0 + dt)]
        stage_cs = [np.float32(0.0), np.float32(dt / 2), np.float32(dt / 2), dt]
        for g, (tg_, cg_) in enumerate(zip(stage_ts, stage_cs)):
            o_g = (t0 - t0_grid + cg_) * b2  # offset of stage-g input state
            b1_eff[s * 4 + g] = b1 + te(tg_) @ W1[2 * D:] + o_g @ W1ha

    # Global pre-tanh scale LAMBDA=16, undone by the tanh's scale=1/16:
    # W1h*16 (bf16); agg path: (W1a*8) fp8 x (agg*2) fp8 -> W1a@agg * 16.
    w1_in = np.ascontiguousarray(
        (W1[:D] * np.float32(16.0)).reshape(2, 128, DT, 128).transpose(1, 0, 2, 3)
    ).astype(ml_dtypes.bfloat16)
    w1a8_in = np.ascontiguousarray(
        (W1[D : 2 * D] * np.float32(8.0))
        .reshape(2, 128, DT, 128).transpose(1, 0, 2, 3)
    ).astype(ml_dtypes.float8_e4m3)
    w2_in = np.ascontiguousarray(
        W2.reshape(DT, 128, D).transpose(1, 0, 2)
    ).astype(ml_dtypes.bfloat16)
    b1_in = np.ascontiguousarray(
        b1_eff.reshape(4 * NSTEP_FULL, DT, 128).transpose(2, 1, 0)
    )
    ident = np.eye(128, dtype=np.float32)

    in_maps = []
    for ci in range(NCORES):
        sl = slice(ci * BPC, (ci + 1) * BPC)
        at_in = (
            adjacency[sl].transpose(0, 2, 1).reshape(BPC, RT, 128, C)
            * np.float32(A_SCALE)
        ).astype(ml_dtypes.float8_e4m3)
        h0_in = rnd(np.ascontiguousarray(
            h0[sl].reshape(BPC, RT, 128, D).transpose(0, 2, 1, 3)))
        in_maps.append(
            {
                "at8": at_in,
                "h0": h0_in,
                "w1": w1_in,
                "w1a8": w1a8_in,
                "w2": w2_in,
                "b1t": b1_in,
                "ident": ident,
            }
        )
    return in_maps, dts[:n_steps]


def _bias_table(time_grid, b2):
    # un-bias: true h = stored h~ + (t_s - t_0) * b2
    tg = np.asarray(time_grid, np.float32)
    return (tg[1:, None] - tg[0]) * np.asarray(b2, np.float32)[None, :]  # [T-1, D]


def gather(results, h0, time_grid, b2, n_steps=NSTEP_FULL):
    h0 = np.asarray(h0, np.float32)
    out = np.empty((B, n_steps + 1, C, D), np.float32)
    out[:, 0] = h0
    bias = _bias_table(time_grid, b2)[:n_steps]  # [n_steps, D]
    for ci in range(NCORES):
        t = results[ci]["traj"]  # [BPC, n_steps, 128, RT, D]
        out[ci * BPC : (ci + 1) * BPC, 1:] = (
            t.transpose(0, 1, 3, 2, 4).reshape(BPC, n_steps, C, D)
            + bias[None, :, None, :]
        )
    return out


def postprocess_core0(traj_core0, ins):
    """sim_run helper: apply the gather-side bias to core-0 output."""
    bias = _bias_table(ins["time_grid"], ins["b2"])  # [T-1, D]
    return traj_core0 + bias[None, :, None, :]


_CACHE = {}


def kernel(h0, time_grid, adjacency, W1, b1, W2, b2):
    from concourse.bass_utils import run_bass_kernel_spmd

    in_maps, dts = host_prep(h0, time_grid, adjacency, W1, b1, W2, b2)
    key = tuple(dts)
    if key not in _CACHE:
        _CACHE[key] = build_program(dts)
    nc = _CACHE[key]
    res = run_bass_kernel_spmd(nc, in_maps, list(range(NCORES)), trace=False)
    return gather(res.results, h0, time_grid, b2)


# revision 6
# speedup vs baseline: 8.9674x; 1.2367x over previous
"""Trainium2 Bass kernel for CLDOdeBlock v2 (fp8 DoubleRow agg + bf16 MLP).

Math (per batch b):
    An = adjacency / max(adjacency.sum(-1, keepdims=True), 1)   (row sums == 1)
    vector_field(t, h) = tanh([h | An@h | te(t)] @ W1 + b1) @ W2 + b2
    RK4 with 8 steps over time_grid; output trajectory [B, T, C, D].

Design (vs the all-f32r baseline):
  - An@h runs in fp8e4 DoubleRow perf mode (0.5 cycles/row, 4x f32r FLOP
    rate): AnT is host-scaled by 512 and stored fp8; h is cast to fp8 on
    device each stage; W1's agg rows are host-scaled by 1/512 to undo. agg
    is a small term (std ~0.04 vs h ~1), so fp8 error on it is negligible.
  - The MLP matmuls (out1, out2) run in bf16 (same 1.0 cycles/row as f32r,
    but evacuations become plain dtype-converting copies that any engine may
    do — no f32r-rounding semantics needed). Only the hT transposes stay
    f32r (their operand is the f32 h state, which IS tf32-rounded because
    the DVE state updates write through f32r-typed APs).
  - b2 is eliminated from the device loop: An is row-stochastic, so constant
    shifts of h pass through aggregation exactly; each stage's b2 shift is
    folded into b1_eff on the host and the stored trajectory is un-biased by
    t_s*b2 in gather().
  - RK4 bookkeeping avoids reading PSUM: with stage states s_g = h + c_g*k_g
    (already computed for the next stage), h_new = u/6 - h/3 + dt/6*k4 where
    u = 2*s1 + 4*s2 + 2*s3. So each p2 PSUM tile has exactly one reader (the
    critical hs update) and its banks recycle immediately; u/z/h' run on DVE
    off the critical path.
  - Phase-major emission (both batches per phase); per-C-quarter hs updates
    so the PE's coalesced wait for the next round's transposes lands just
    after the last quarter.
  - Engine split per eval: PE matmuls; ACT fp8 casts + tanh; Pool hT/agT
    evacuations; DVE state math.
"""

import math
from contextlib import ExitStack, nullcontext

import numpy as np

import concourse.bass as bass
import concourse.tile as tile
from concourse import bacc, mybir
from concourse.bass import ds

B, C, D = 16, 1024, 256
T = 9
NSTEP_FULL = T - 1
NCORES = 8
BPC = B // NCORES  # batches per core
TIME_DIM = 32
HALF = TIME_DIM // 2
F32 = mybir.dt.float32
F32R = mybir.dt.float32r
BF16 = mybir.dt.bfloat16
FP8 = mybir.dt.float8e4

RT = C // 128   # 8 row tiles
DT = D // 128   # 2 feature tiles
NH = C // 512   # 2 free halves for N=512 matmuls
JP = RT // 2    # 4 DoubleRow k-tile pairs
RH = RT // 2    # row tiles per C-half
QD = RT // 4    # row tiles per C-quarter

A_SCALE = 512.0  # host scale on AnT (fp8 range); W1 agg rows divided by it


def build_program(dts, n_steps=NSTEP_FULL, n_iters=1, use_f32r=True,
                  skip_stores=False):
    """Build + compile the per-core Bass program.

    dts: python floats, len n_steps (the RK4 dt per step; baked in).
    n_iters: >1 wraps the whole computation in a For_i loop (for timing).
    """
    nc = bacc.Bacc("TRN2", target_bir_lowering=False, debug=False)

    at_d = nc.dram_tensor("at8", [BPC, RT, 128, C], FP8, kind="ExternalInput").ap()
    h0_d = nc.dram_tensor("h0", [BPC, 128, RT, D], F32, kind="ExternalInput").ap()
    w1_d = nc.dram_tensor("w1", [128, 2, DT, 128], BF16, kind="ExternalInput").ap()
    w1a8_d = nc.dram_tensor("w1a8", [128, 2, DT, 128], FP8, kind="ExternalInput").ap()
    w2_d = nc.dram_tensor("w2", [128, DT, D], BF16, kind="ExternalInput").ap()
    b1_d = nc.dram_tensor("b1t", [128, DT, 4 * NSTEP_FULL], F32, kind="ExternalInput").ap()
    id_d = nc.dram_tensor("ident", [128, 128], F32, kind="ExternalInput").ap()
    tr_d = nc.dram_tensor("traj", [BPC, n_steps, 128, RT, D], F32, kind="ExternalOutput").ap()

    def mm(ap):
        return ap.bitcast(F32R) if use_f32r else ap

    DR = mybir.MatmulPerfMode.DoubleRow

    with ExitStack() as ctx:
        tc = ctx.enter_context(tile.TileContext(nc))
        const = ctx.enter_context(tc.tile_pool(name="const", bufs=1))
        at_p = ctx.enter_context(tc.tile_pool(name="atp", bufs=1))

        # ---- constants / weights ----
        at_sb = at_p.tile([128, BPC, RT, C], FP8)
        for b in range(BPC):
            for jc in range(RT):
                nc.sync.dma_start(at_sb[:, b, jc, :], at_d[b, jc])
        w1_sb = const.tile([128, 2, DT, 128], BF16)
        nc.sync.dma_start(w1_sb[:], w1_d)
        w1a8_sb = const.tile([128, 2, DT, 128], FP8)
        nc.sync.dma_start(w1a8_sb[:], w1a8_d)
        w2_sb = const.tile([128, DT, D], BF16)
        nc.sync.dma_start(w2_sb[:], w2_d)
        b1_sb = const.tile([128, DT, 4 * NSTEP_FULL], F32)
        nc.sync.dma_start(b1_sb[:], b1_d)
        id_sb = const.tile([128, 128], F32)
        nc.sync.dma_start(mm(id_sb[:]), mm(id_d))

        # ---- main pools ----
        state_p = ctx.enter_context(tc.tile_pool(name="state", bufs=4))
        hs_p = ctx.enter_context(tc.tile_pool(name="hs", bufs=2))
        bk_p = ctx.enter_context(tc.tile_pool(name="bk", bufs=2))
        tp_p = ctx.enter_context(tc.tile_pool(name="tp", bufs=4))
        h8_p = ctx.enter_context(tc.tile_pool(name="h8", bufs=3))
        ps_p = ctx.enter_context(tc.tile_pool(name="ps", bufs=4, space="PSUM"))

        loop_cm = tc.For_i(0, n_iters) if n_iters > 1 else nullcontext()
        with loop_cm:
            hstates = []
            for b in range(BPC):
                hst = state_p.tile([128, RT, D], F32, tag="hst")
                nc.sync.dma_start(mm(hst[:]), mm(h0_d[b]))
                hstates.append(hst)
            hstage = [None] * BPC
            pending_bk = []
            zneg = [None] * BPC
            hprime = [None] * BPC

            for s in range(n_steps):
                dt = float(dts[s])
                for g in range(4):
                    ev = s * 4 + g
                    h_in = [hstates[b] if g == 0 else hstage[b] for b in range(BPC)]

                    # --- phase 0: fp8 cast of h, per C-half on Pool
                    # (SBUF->SBUF; Pool cannot access PSUM). The previous
                    # eval's bookkeeping is emitted AFTER these casts so the
                    # Pool queue never head-of-line blocks the agg matmuls. ---
                    h8s = []
                    for b in range(BPC):
                        h8 = h8_p.tile([128, RT, D], FP8, tag="h8")
                        nc.scalar.copy(h8[:, :RH], h_in[b][:, :RH])
                        nc.scalar.copy(h8[:, RH:], h_in[b][:, RH:])
                        h8s.append(h8)
                    for fn in pending_bk:
                        fn()
                    pending_bk = []

                    # --- phase 1: hT transposes (PE, f32r, jc-major), evac
                    # to bf16 on Pool ---
                    hTs = []
                    for b in range(BPC):
                        hT = tp_p.tile([128, DT, C], BF16, tag="tp")
                        pts = [
                            ps_p.tile([128, C], F32, tag="ps", name=f"pt{d_}")
                            for d_ in range(DT)
                        ]
                        for jc in range(RT):
                            for d_ in range(DT):
                                nc.tensor.transpose(
                                    mm(pts[d_][:, ds(jc * 128, 128)]),
                                    mm(h_in[b][:, jc, ds(d_ * 128, 128)]),
                                    mm(id_sb[:]),
                                )
                        for d_ in range(DT):
                            nc.scalar.copy(hT[:, d_, :], pts[d_][:])
                        hTs.append(hT)

                    # --- phase 2: aggT via fp8 DoubleRow (PE), evac to fp8
                    # (x1/256 => agg*2) on DVE ---
                    agTs = []
                    for b in range(BPC):
                        agT = tp_p.tile([128, DT, C], FP8, tag="tp8")
                        for d_ in range(DT):
                            pa = ps_p.tile([128, C], F32, tag="ps")
                            for jp in range(JP):
                                for nh in range(NH):
                                    nc.tensor.matmul(
                                        pa[:, ds(nh * 512, 512)],
                                        h8s[b][:, ds(2 * jp, 2), ds(d_ * 128, 128)],
                                        at_sb[:, b, ds(2 * jp, 2), ds(nh * 512, 512)],
                                        start=(jp == 0),
                                        stop=(jp == JP - 1),
                                        perf_mode=DR,
                                    )
                            nc.vector.tensor_scalar_mul(
                                agT[:, d_, :], pa[:], 1.0 / 256.0
                            )
                        agTs.append(agT)

                    # --- phase 3: out1 (PE, bf16) + tanh w/ folded bias (ACT,
                    # bf16 out) ---
                    a1s = []
                    for b in range(BPC):
                        a1 = tp_p.tile([128, DT, C], BF16, tag="tp")
                        for ht in range(DT):
                            p1 = ps_p.tile([128, C], F32, tag="ps")
                            for kc in range(2):
                                for nh in range(NH):
                                    nc.tensor.matmul(
                                        p1[:, ds(nh * 512, 512)],
                                        w1_sb[:, kc, ht, :],
                                        hTs[b][:, kc, ds(nh * 512, 512)],
                                        start=(kc == 0),
                                        stop=False,
                                        skip_group_check=True,
                                    )
                            for nh in range(NH):
                                nc.tensor.matmul(
                                    p1[:, ds(nh * 512, 512)],
                                    w1a8_sb[:, :, ht, :],
                                    agTs[b][:, :, ds(nh * 512, 512)],
                                    start=False,
                                    stop=True,
                                    perf_mode=DR,
                                    skip_group_check=True,
                                )
                            nc.scalar.activation(
                                a1[:, ht, :],
                                p1[:],
                                mybir.ActivationFunctionType.Tanh,
                                bias=b1_sb[:, ht, ev : ev + 1],
                                scale=1.0 / 16.0,
                            )
                        a1s.append(a1)

                    # --- phase 4: out2 (PE, bf16) + critical hs updates (DVE,
                    # per C-quarter, sole readers of p2) ---
                    hss = []
                    for b in range(BPC):
                        if g < 3:
                            hs = hs_p.tile([128, RT, D], F32, tag="hs")
                        else:
                            hs = state_p.tile([128, RT, D], F32, tag="hst")
                        hss.append(hs)
                        p2s = []
                        for rh in range(2):
                            p2 = ps_p.tile([128, RH, D], F32, tag="ps")
                            p2s.append(p2)
                            for r4 in range(RH):
                                rt = rh * RH + r4
                                for kc in range(DT):
                                    nc.tensor.matmul(
                                        p2[:, r4, :],
                                        a1s[b][:, kc, ds(rt * 128, 128)],
                                        w2_sb[:, kc, :],
                                        start=(kc == 0),
                                        stop=(kc == DT - 1),
                                    )
                        cs = (dt / 2 if g < 2 else dt) if g < 3 else dt / 6.0
                        base = hstates[b] if g < 3 else hprime[b]
                        for rh in range(2):
                            hsl = ds(rh * RH, RH)
                            nc.vector.scalar_tensor_tensor(
                                mm(hs[:, hsl]), p2s[rh][:], cs, base[:, hsl],
                                mybir.AluOpType.mult, mybir.AluOpType.add,
                            )

                    # --- phase 5: bookkeeping on DVE (off critical path,
                    # emission deferred to the next eval's phase 0):
                    # h' = (s1 + 2*s2 + s3 - h)/3 built in z via stt ops.
                    for b in range(BPC):
                        hs = hss[b]
                        if g == 0:
                            z = bk_p.tile([128, RT, D], F32, tag="z")
                            zneg[b] = z

                            def bk0(z=z, hs=hs, hst=hstates[b]):
                                nc.vector.scalar_tensor_tensor(
                                    z[:], hst[:], -1.0, hs[:],
                                    mybir.AluOpType.mult, mybir.AluOpType.add,
                                )

                            pending_bk.append(bk0)
                        elif g == 1:

                            def bk1(z=zneg[b], hs=hs):
                                nc.vector.scalar_tensor_tensor(
                                    z[:], hs[:], 2.0, z[:],
                                    mybir.AluOpType.mult, mybir.AluOpType.add,
                                )

                            pending_bk.append(bk1)
                        elif g == 2:
                            hp = bk_p.tile([128, RT, D], F32, tag="hp")
                            hprime[b] = hp

                            def bk2(z=zneg[b], hs=hs, hp=hp):
                                nc.vector.scalar_tensor_tensor(
                                    hp[:], hs[:], 1.0, z[:],
                                    mybir.AluOpType.mult, mybir.AluOpType.add,
                                )
                                nc.vector.tensor_scalar_mul(
                                    hp[:], hp[:], 1.0 / 3.0
                                )

                            pending_bk.append(bk2)
                        else:
                            hstates[b] = hss[b]
                            if not skip_stores:
                                nc.sync.dma_start(tr_d[b, s], hss[b][:])
                        if g < 3:
                            hstage[b] = hss[b]

    nc.compile()
    return nc


def tf32_round(x):
    """Round-to-nearest-even to TF32 (10 mantissa bits)."""
    u = np.ascontiguousarray(x, np.float32).view(np.uint32)
    lsb = (u >> np.uint32(13)) & np.uint32(1)
    u = u + np.uint32(0x0FFF) + lsb
    u &= np.uint32(0xFFFFE000)
    return u.view(np.float32)


def host_prep(h0, time_grid, adjacency, W1, b1, W2, b2, n_steps=NSTEP_FULL,
              use_f32r=True):
    """Returns (in_maps list per core, dts list)."""
    import ml_dtypes

    h0 = np.asarray(h0, np.float32)
    time_grid = np.asarray(time_grid, np.float32)
    adjacency = np.asarray(adjacency, np.float32)
    W1 = np.asarray(W1, np.float32)
    b1 = np.asarray(b1, np.float32)
    W2 = np.asarray(W2, np.float32)
    b2 = np.asarray(b2, np.float32)

    # degree normalization (time-constant input prep)
    deg = np.maximum(adjacency.sum(-1, keepdims=True), np.float32(1.0))
    # the b2 fold below needs exact row sums of 1 (degree > 1 everywhere)
    assert adjacency.sum(-1).min() > 1.0
    adjacency = (adjacency / deg).astype(np.float32)

    rnd = tf32_round if use_f32r else (lambda x: x)

    # Fourier features folded into per-eval bias
    freqs = np.exp(
        -math.log(10000.0) * np.arange(HALF, dtype=np.float32) / np.float32(HALF)
    ).astype(np.float32)

    def te(t):
        a = (np.float32(t) * freqs).astype(np.float32)
        return np.concatenate([np.sin(a), np.cos(a)]).astype(np.float32)

    # b2 fold: stored state h~ = h - o(s), o(s) = (t_s - t_0) * b2. Constant
    # shifts pass through the row-stochastic aggregation exactly, so each
    # stage's pre-tanh shift is o_g @ (W1h + W1a), folded into b1_eff.
    W1ha = (W1[:D] + W1[D : 2 * D]).astype(np.float32)  # [D, D]
    t0_grid = np.float32(time_grid[0])

    dts = []
    b1_eff = np.zeros((4 * NSTEP_FULL, D), np.float32)
    for s in range(NSTEP_FULL):
        t0 = np.float32(time_grid[s])
        t1 = np.float32(time_grid[s + 1])
        dt = np.float32(t1 - t0)
        dts.append(float(dt))
        stage_ts = [t0, np.float32(t0 + dt / 2), np.float32(t0 + dt / 2),
                    np.float32(t0 + dt)]
        stage_cs = [np.float32(0.0), np.float32(dt / 2), np.float32(dt / 2), dt]
        for g, (tg_, cg_) in enumerate(zip(stage_ts, stage_cs)):
            o_g = (t0 - t0_grid + cg_) * b2  # offset of stage-g input state
            b1_eff[s * 4 + g] = b1 + te(tg_) @ W1[2 * D:] + o_g @ W1ha

    # Global pre-tanh scale LAMBDA=16, undone by the tanh's scale=1/16:
    # W1h*16 (bf16); agg path: (W1a*8) fp8 x (agg*2) fp8 -> W1a@agg * 16.
    w1_in = np.ascontiguousarray(
        (W1[:D] * np.float32(16.0)).reshape(2, 128, DT, 128).transpose(1, 0, 2, 3)
    ).astype(ml_dtypes.bfloat16)
    w1a8_in = np.ascontiguousarray(
        (W1[D : 2 * D] * np.float32(8.0))
        .reshape(2, 128, DT, 128).transpose(1, 0, 2, 3)
    ).astype(ml_dtypes.float8_e4m3)
    w2_in = np.ascontiguousarray(
        W2.reshape(DT, 128, D).transpose(1, 0, 2)
    ).astype(ml_dtypes.bfloat16)
    b1_in = np.ascontiguousarray(
        b1_eff.reshape(4 * NSTEP_FULL, DT, 128).transpose(2, 1, 0)
    )
    ident = np.eye(128, dtype=np.float32)

    in_maps = []
    for ci in range(NCORES):
        sl = slice(ci * BPC, (ci + 1) * BPC)
        at_in = (
            adjacency[sl].transpose(0, 2, 1).reshape(BPC, RT, 128, C)
            * np.float32(A_SCALE)
        ).astype(ml_dtypes.float8_e4m3)
        h0_in = rnd(np.ascontiguousarray(
            h0[sl].reshape(BPC, RT, 128, D).transpose(0, 2, 1, 3)))
        in_maps.append(
            {
                "at8": at_in,
                "h0": h0_in,
                "w1": w1_in,
                "w1a8": w1a8_in,
                "w2": w2_in,
                "b1t": b1_in,
                "ident": ident,
            }
        )
    return in_maps, dts[:n_steps]


def _bias_table(time_grid, b2):
    # un-bias: true h = stored h~ + (t_s - t_0) * b2
    tg = np.asarray(time_grid, np.float32)
    return (tg[1:, None] - tg[0]) * np.asarray(b2, np.float32)[None, :]  # [T-1, D]


def gather(results, h0, time_grid, b2, n_steps=NSTEP_FULL):
    h0 = np.asarray(h0, np.float32)
    out = np.empty((B, n_steps + 1, C, D), np.float32)
    out[:, 0] = h0
    bias = _bias_table(time_grid, b2)[:n_steps]  # [n_steps, D]
    for ci in range(NCORES):
        t = results[ci]["traj"]  # [BPC, n_steps, 128, RT, D]
        out[ci * BPC : (ci + 1) * BPC, 1:] = (
            t.transpose(0, 1, 3, 2, 4).reshape(BPC, n_steps, C, D)
            + bias[None, :, None, :]
        )
    return out


def postprocess_core0(traj_core0, ins):
    """sim_run helper: apply the gather-side bias to core-0 output."""
    bias = _bias_table(ins["time_grid"], ins["b2"])  # [T-1, D]
    return traj_core0 + bias[None, :, None, :]


_CACHE = {}


def kernel(h0, time_grid, adjacency, W1, b1, W2, b2):
    from concourse.bass_utils import run_bass_kernel_spmd

    in_maps, dts = host_prep(h0, time_grid, adjacency, W1, b1, W2, b2)
    key = tuple(dts)
    if key not in _CACHE:
        _CACHE[key] = build_program(dts)
    nc = _CACHE[key]
    res = run_bass_kernel_spmd(nc, in_maps, list(range(NCORES)), trace=False)
    return gather(res.results, h0, time_grid, b2)
